# revision 5
# baseline (speedup 1.0000x reference)
"""Multi-head attention (B=4, S=2048, D=512, H=8) on 8 Trainium2 NeuronCores.

Sharding: core c handles batch b = c//2 and head-group hg = c%2 (4 heads,
256 of the 512 output dims). No cross-core communication is needed: each
core computes out[b, :, hg*256:(hg+1)*256] fully.

Device-side layout strategy (per core):
  - host passes x[b] pre-transposed (xT [D, S]) plus head-group weight
    slices pre-transposed (w* [D_in=512, D_out=256]) so the kernel never
    transposes activations or weights on device.
  - projections compute qT/kT in [d, s] layout (bias added per-partition on
    DVE) and v in natural [t, d] layout with an extra all-ones column per
    head; the ones column makes the PV matmul emit the softmax denominator
    as output row 64 for free.
  - scores are computed transposed (scoresT[t, s]) so softmax's exp can run
    on ScalarE straight out of PSUM into SBUF (the exp IS the PSUM->SBUF
    move) and the PV matmul consumes probsT with no transposes.
  - exp uses the activation instruction's free affine to apply the 1/8
    attention scale; no max-subtraction is needed (|scores|/8 <= ~15 in
    fp32).
  - the [65, s] PV result is PE-transposed per 128-row block, normalized by
    the denominator column with a per-partition reciprocal multiply, and the
    v-bias (broadcast tile from host) is added at the end:
        out = (probs_unnorm @ v_nobias) / denom + bv
"""

import sys

for _p in ("/opt/trn_rl_repo", "/root/.axon_site/_ro/trn_rl_repo"):
    if _p not in sys.path:
        sys.path.insert(0, _p)

import numpy as np

import bass_rust
import concourse.bass as bass
import concourse.tile as tile
from concourse import mybir
from concourse.bass_utils import run_bass_kernel_spmd

B, S, D = 4, 2048, 512
H = 8
HD = D // H  # 64
N_CORES = 8
HEADS_PER_CORE = 4
DC = HEADS_PER_CORE * HD  # 256 output dims per core
F32 = mybir.dt.float32
F32R = mybir.dt.float32r

KC = D // 128  # 4 contraction chunks for projections
MC = DC // 128  # 2 output-partition chunks for q/k projections
TB = S // 128  # 16 t blocks
SC = S // 1024  # 2 s-chunks of 1024
VW = HD + 1  # 65: per-head v width incl. ones column


def _split_multi_waits(nc, max_waits=1):
    """This walrus build accepts at most one sync wait per instruction;
    Tile emits up to two. Move extra waits onto nop instructions inserted
    just before the offending instruction on the same engine."""
    n_split = 0
    for bb in nc.main_func.blocks:
        new_list = []
        changed = False
        for inst in bb.instructions:
            si = inst.sync_info
            if si is not None and len(si.on_wait) > max_waits:
                waits = list(si.on_wait)
                for w in waits[max_waits:]:
                    nop = bass_rust.InstNoOp(
                        name=nc.get_next_instruction_name(), ins=[], outs=[]
                    )
                    nop.engine = inst.engine
                    nop.sync_info = bass_rust.SyncInfo(
                        on_wait=[w], on_update=[]
                    )
                    nc.register_instruction(nop, overwrite=True)
                    new_list.append(nop)
                inst.sync_info = bass_rust.SyncInfo(
                    on_wait=waits[:max_waits], on_update=list(si.on_update)
                )
                changed = True
                n_split += 1
            new_list.append(inst)
        if changed:
            bb.instructions = new_list
    return n_split


def _r(ap):
    return ap.bitcast(F32R)


def build_program() -> bass.Bass:
    nc = bass.Bass("TRN2", target_bir_lowering=False, debug=False,
                   num_devices=N_CORES)

    xT = nc.declare_dram_parameter("xT", [D, S], F32, isOutput=False).ap()
    wq = nc.declare_dram_parameter("wq", [D, DC], F32, isOutput=False).ap()
    wk = nc.declare_dram_parameter("wk", [D, DC], F32, isOutput=False).ap()
    wv = nc.declare_dram_parameter("wv", [D, DC], F32, isOutput=False).ap()
    bq2 = nc.declare_dram_parameter("bq2", [128, MC], F32, isOutput=False).ap()
    bk2 = nc.declare_dram_parameter("bk2", [128, MC], F32, isOutput=False).ap()
    bvb = nc.declare_dram_parameter("bvb", [128, DC], F32, isOutput=False).ap()
    ident = nc.declare_dram_parameter("ident", [128, 128], F32,
                                      isOutput=False).ap()
    out = nc.declare_dram_parameter("out", [S, DC], F32, isOutput=True).ap()

    xT_r = xT.rearrange("(k p) s -> k p s", p=128)
    wq_r = wq.rearrange("(k p) m -> k p m", p=128)
    wk_r = wk.rearrange("(k p) m -> k p m", p=128)
    wv_r = wv.rearrange("(k p) m -> k p m", p=128)

    with tile.TileContext(nc) as tc:
        with (
            tc.tile_pool(name="const", bufs=1) as const,
            tc.tile_pool(name="acts", bufs=1) as acts,
            tc.tile_pool(name="probs", bufs=3) as probs_pool,
            tc.tile_pool(name="osb", bufs=2) as osb_pool,
            tc.tile_pool(name="small", bufs=4) as small_pool,
            tc.tile_pool(name="psA", bufs=2, space="PSUM") as psA,
            tc.tile_pool(name="psO", bufs=1, space="PSUM") as psO,
            tc.tile_pool(name="psT", bufs=2, space="PSUM") as psT,
        ):
            # ---- constants / inputs to SBUF ----
            # matmul operands must be explicitly rounded to fp32r by their
            # producer (BIR verifier rule), so DMA-loaded tensors get a DVE
            # rounding copy into an fp32r-typed tile.
            xt_sb = []
            for k in range(KC):
                t0 = const.tile([128, S], F32, tag=f"xtf{k}", name=f"xtf{k}")
                nc.sync.dma_start(out=t0, in_=xT_r[k])
                t = const.tile([128, S], F32R, tag=f"xt{k}", name=f"xt{k}")
                nc.vector.tensor_copy(out=t, in_=t0)
                xt_sb.append(t)
            w_sb = {}
            for name, ap_r in (("q", wq_r), ("k", wk_r), ("v", wv_r)):
                for k in range(KC):
                    t0 = const.tile([128, DC], F32, tag=f"wf{name}{k}", name=f"wf{name}{k}")
                    nc.sync.dma_start(out=t0, in_=ap_r[k])
                    t = const.tile([128, DC], F32R, tag=f"w{name}{k}", name=f"w{name}{k}")
                    nc.vector.tensor_copy(out=t, in_=t0)
                    w_sb[name, k] = t
            bq_sb = const.tile([128, MC], F32, tag="bq", name="bq")
            nc.sync.dma_start(out=bq_sb, in_=bq2)
            bk_sb = const.tile([128, MC], F32, tag="bk", name="bk")
            nc.sync.dma_start(out=bk_sb, in_=bk2)
            bvb_sb = const.tile([128, DC], F32, tag="bvb", name="bvb")
            nc.sync.dma_start(out=bvb_sb, in_=bvb)
            id_sb = const.tile([128, 128], F32, tag="ident", name="ident")
            nc.sync.dma_start(out=id_sb, in_=ident)

            # ---- projections ----
            # qT/kT: [128, S] per m-chunk (two heads stacked per tile)
            qkt_sb = {}
            for name, b_sb in (("q", bq_sb), ("k", bk_sb)):
                for m in range(MC):
                    dst = acts.tile([128, S], F32R, tag=f"{name}T{m}", name=f"{name}T{m}")
                    qkt_sb[name, m] = dst
                    for n in range(S // 512):
                        ps = psA.tile([128, 512], F32, tag="big", name="big")
                        for k in range(KC):
                            nc.tensor.matmul(
                                ps,
                                lhsT=w_sb[name, k][:, m * 128:(m + 1) * 128],
                                rhs=xt_sb[k][:, n * 512:(n + 1) * 512],
                                start=(k == 0),
                                stop=(k == KC - 1),
                            )
                        nc.vector.tensor_scalar_add(
                            out=dst[:, n * 512:(n + 1) * 512],
                            in0=ps,
                            scalar1=b_sb[:, m:m + 1],
                        )

            # v natural [t, d] with a ones column per head: [128, 4*65]
            ones4 = const.tile([128, HEADS_PER_CORE], F32, tag="ones4",
                               name="ones4")
            nc.vector.memset(ones4, 1.0)
            ones4_v = ones4.rearrange("p (h o) -> p h o", o=1)
            vaug_sb = []
            for tb in range(TB):
                vt = acts.tile([128, HEADS_PER_CORE * VW], F32R, tag=f"vaug{tb}", name=f"vaug{tb}")
                vaug_sb.append(vt)
                vt_view = vt.rearrange("p (h e) -> p h e", e=VW)
                nc.vector.tensor_copy(out=vt_view[:, :, HD:VW], in_=ones4_v)
                ps = psA.tile([128, DC], F32, tag="big", name="big")
                for k in range(KC):
                    nc.tensor.matmul(
                        ps,
                        lhsT=xt_sb[k][:, tb * 128:(tb + 1) * 128],
                        rhs=w_sb["v", k],
                        start=(k == 0),
                        stop=(k == KC - 1),
                    )
                nc.vector.tensor_copy(
                    out=vt_view[:, :, 0:HD],
                    in_=ps.rearrange("p (h e) -> p h e", e=HD),
                )

            # ---- output assembly tiles (one per 128-row s block) ----
            asm = [acts.tile([128, DC], F32, tag=f"asm{i}", name=f"asm{i}") for i in range(TB)]

            # ---- attention ----
            for h in range(HEADS_PER_CORE):
                m = h // 2
                p0 = (h % 2) * 64
                kT = qkt_sb["k", m]
                qT = qkt_sb["q", m]
                for sc in range(SC):
                    outp = psO.tile([VW, 1024], F32, tag="out", name="out")
                    for tb in range(TB):
                        sp = psA.tile([128, 1024], F32, tag="big", name="big")
                        for j in range(2):
                            s_off = sc * 1024 + j * 512
                            nc.tensor.matmul(
                                sp[:, j * 512:(j + 1) * 512],
                                lhsT=kT[p0:p0 + 64, tb * 128:(tb + 1) * 128],
                                rhs=qT[p0:p0 + 64, s_off:s_off + 512],
                                start=True,
                                stop=True,
                            )
                        pr = probs_pool.tile([128, 1024], F32R, tag="pr", name="pr")
                        nc.scalar.activation(
                            out=pr, in_=sp,
                            func=mybir.ActivationFunctionType.Exp,
                            scale=0.125,
                        )
                        for j in range(2):
                            nc.tensor.matmul(
                                outp[:, j * 512:(j + 1) * 512],
                                lhsT=vaug_sb[tb][:, h * VW:(h + 1) * VW],
                                rhs=pr[:, j * 512:(j + 1) * 512],
                                start=(tb == 0),
                                stop=(tb == TB - 1),
                            )
                    # epilogue: transpose to [s, 65], normalize, add bias
                    osb = osb_pool.tile([VW, 1024], F32, tag="osb", name="osb")
                    nc.vector.tensor_copy(out=osb, in_=outp)
                    for sb in range(8):
                        tp = psT.tile([128, VW], F32, tag="tp", name="tp")
                        nc.tensor.transpose(
                            out=tp,
                            in_=osb[:, sb * 128:(sb + 1) * 128],
                            identity=id_sb[0:VW, 0:VW],
                        )
                        rec = small_pool.tile([128, 1], F32, tag="rec", name="rec")
                        nc.vector.reciprocal(out=rec, in_=tp[:, HD:VW])
                        a = asm[sc * 8 + sb]
                        nc.vector.tensor_scalar_mul(
                            out=a[:, h * HD:(h + 1) * HD],
                            in0=tp[:, 0:HD],
                            scalar1=rec,
                        )
                        nc.vector.tensor_add(
                            out=a[:, h * HD:(h + 1) * HD],
                            in0=a[:, h * HD:(h + 1) * HD],
                            in1=bvb_sb[:, h * HD:(h + 1) * HD],
                        )

            for i in range(TB):
                nc.sync.dma_start(out=out[i * 128:(i + 1) * 128, :], in_=asm[i])

    _split_multi_waits(nc)
    return nc


_PROGRAM_CACHE = {}


def _get_program():
    if "nc" not in _PROGRAM_CACHE:
        _PROGRAM_CACHE["nc"] = build_program()
    return _PROGRAM_CACHE["nc"]


def make_in_maps(x, Wq, bq, Wk, bk, Wv, bv):
    in_maps = []
    ident = np.eye(128, dtype=np.float32)
    for c in range(N_CORES):
        b = c // 2
        hg = c % 2
        sl = slice(hg * DC, (hg + 1) * DC)
        in_maps.append({
            "xT": np.ascontiguousarray(x[b].T),
            "wq": np.ascontiguousarray(Wq[sl, :].T),
            "wk": np.ascontiguousarray(Wk[sl, :].T),
            "wv": np.ascontiguousarray(Wv[sl, :].T),
            "bq2": np.ascontiguousarray(bq[sl].reshape(MC, 128).T),
            "bk2": np.ascontiguousarray(bk[sl].reshape(MC, 128).T),
            "bvb": np.tile(bv[sl][None, :], (128, 1)).astype(np.float32),
            "ident": ident,
        })
    return in_maps


def gather_output(results):
    out = np.empty((B, S, D), dtype=np.float32)
    for c in range(N_CORES):
        b = c // 2
        hg = c % 2
        out[b, :, hg * DC:(hg + 1) * DC] = results[c]["out"]
    return out


def kernel(x, Wq, bq, Wk, bk, Wv, bv, **run_kwargs):
    x = np.asarray(x, dtype=np.float32)
    nc = _get_program()
    in_maps = make_in_maps(np.asarray(x), np.asarray(Wq), np.asarray(bq),
                           np.asarray(Wk), np.asarray(bk), np.asarray(Wv),
                           np.asarray(bv))
    res = run_bass_kernel_spmd(nc, in_maps, list(range(N_CORES)), **run_kwargs)
    out = gather_output(res.results)
    if run_kwargs:
        return out, res
    return out
